# revision 13
# baseline (speedup 1.0000x reference)
"""Trainium2 Bass kernel for causal multi-head attention with positional bias.

Reference computation (B=4, N=2048, D=1024, H=16, dh=64):
    qkv = x @ w_qkv; q,k,v = split(qkv); q *= dh**-0.5
    sim = q @ k.T + pos_bias; causal mask; attn = softmax(sim)
    out = (attn @ v) @ w_proj + b_proj

Sharding over 8 NeuronCores: core c handles batch c//2 and heads
8*(c%2) .. 8*(c%2)+8.  Each core computes its heads' full causal
attention in transposed layout (S_T[k,q] = K @ Q^T) so no on-chip
transposes are needed, then the partial output projection; a pair-wise
ReduceScatter (cores 2b, 2b+1) sums the head-sharded projection and
splits the output-feature dim.

Tricks:
- exp(bias) is precomputed on the host (masked entries = 0), so the bias
  add + causal mask become one bf16 multiply: P = exp(S) * EB.
- V carries an appended ones column, so the AV matmul also produces the
  softmax denominator for free.
- Softmax division: batched reciprocal of all 8 head denominators, then
  a partition-broadcast DMA (via DRAM) and one multiply per head.
- b_proj/2 is folded into the projection's PSUM->SBUF copy; after the
  pair ReduceScatter the halves sum to exactly b_proj.
- Causal width trimming: blocks above the diagonal are skipped entirely;
  the diagonal pair of key chunks only computes the valid query suffix.

Self-contained: hardcodes all shapes; no file reads.
"""

import numpy as np
import ml_dtypes

import concourse.bass as bass
import concourse.tile as tile
from concourse import bacc, mybir
from concourse.bass_utils import run_bass_kernel_spmd

F32 = mybir.dt.float32
F32R = mybir.dt.float32r
BF16 = mybir.dt.bfloat16

PAIRS = [[0, 1], [2, 3], [4, 5], [6, 7]]


class Cfg:
    """Geometry. Defaults = the real problem; small variants for sim tests."""

    def __init__(self, n=2048, d_in=1024, hl=8, d_out=1024):
        self.P = 128
        self.STRIP = 512          # query columns per strip (psum bank width)
        self.n = n                # sequence length
        self.d_in = d_in          # model dim
        self.hl = hl              # local heads per core
        self.dh = 64              # head dim
        self.d_out = d_out        # proj output dim
        self.nstrip = n // self.STRIP
        self.nkc = n // self.P            # key chunks of 128
        self.kcd = d_in // self.P         # contraction chunks over d_in
        self.hp = hl // 2                 # head pairs
        self.qk_cols = hl * self.dh       # local q (or k, v) columns
        self.oc = d_out // self.P         # out-feature chunks
        self.rs_rows = d_out // 2         # rows each core owns after RS
        kps = self.STRIP // self.P        # 128-row chunks per strip (4)
        self.kps = kps
        self.bias_off = [0] * self.nstrip
        off = 0
        for j in range(self.nstrip):
            self.bias_off[j] = off
            off += (j + 1) * self.STRIP
        self.bias_rows = off              # 5120 for full size


def build_nc(cfg: Cfg, num_devices=8, debug=False):
    P, STRIP = cfg.P, cfg.STRIP
    nc = bacc.Bacc("TRN2", target_bir_lowering=False, debug=debug,
                   num_devices=num_devices)

    xT_e = nc.dram_tensor("xT", [cfg.d_in, cfg.n], BF16, kind="ExternalInput")
    wq_e = nc.dram_tensor("wq", [cfg.d_in, cfg.qk_cols], BF16, kind="ExternalInput")
    wk_e = nc.dram_tensor("wk", [cfg.d_in, cfg.qk_cols], BF16, kind="ExternalInput")
    wv_e = nc.dram_tensor("wv", [cfg.d_in, cfg.qk_cols], BF16, kind="ExternalInput")
    bias_e = nc.dram_tensor("bias", [cfg.hl, cfg.bias_rows, STRIP], BF16,
                            kind="ExternalInput")
    wp_e = nc.dram_tensor("wproj", [cfg.qk_cols, cfg.d_out], F32R,
                          kind="ExternalInput")
    bp_e = nc.dram_tensor("bp", [P, cfg.oc], F32, kind="ExternalInput")
    out_e = nc.dram_tensor("out", [cfg.rs_rows, cfg.n], F32, kind="ExternalOutput")

    partialT = [nc.dram_tensor(f"partialT{j}", [cfg.d_out, STRIP], F32)
                for j in range(cfg.nstrip)]
    rs_out = [nc.dram_tensor(f"rs_out{j}", [cfg.rs_rows, STRIP], F32)
              for j in range(cfg.nstrip)]
    rec_dram = [nc.dram_tensor(f"rec{j}", [cfg.hl, STRIP], F32)
                for j in range(cfg.nstrip)]

    Exp = mybir.ActivationFunctionType.Exp

    with tile.TileContext(nc) as tc:
        with (
            tc.tile_pool(name="qk", bufs=1) as qk_pool,
            tc.tile_pool(name="vp", bufs=1) as v_pool,
            tc.tile_pool(name="eb", bufs=4) as eb_pool,
            tc.tile_pool(name="pp", bufs=6) as p_pool,
            tc.tile_pool(name="oo", bufs=2) as o_pool,
            tc.tile_pool(name="orw", bufs=1) as or_pool,
            tc.tile_pool(name="w2", bufs=1) as w2_pool,
            tc.tile_pool(name="misc", bufs=1) as misc_pool,
            tc.tile_pool(name="rp", bufs=2) as rp_pool,
            tc.tile_pool(name="bcp", bufs=3) as bc_pool,
            tc.tile_pool(name="fin", bufs=3) as fin_pool,
            tc.tile_pool(name="psM", bufs=2, space="PSUM") as psM,
            tc.tile_pool(name="psS", bufs=2, space="PSUM") as psS,
            tc.tile_pool(name="psO", bufs=2, space="PSUM") as psO,
        ):
            # ---------------- phase 1: QKV ----------------
            qT = []
            kT = []
            vsb = []
            with tc.tile_pool(name="xw", bufs=1) as xw_pool:
                xt = xw_pool.tile([P, cfg.kcd, cfg.n], BF16, tag="xt")
                for kc in range(cfg.kcd):
                    nc.sync.dma_start(out=xt[:, kc, :],
                                      in_=xT_e[kc * P:(kc + 1) * P, :])
                wq = xw_pool.tile([P, cfg.kcd, cfg.qk_cols], BF16, tag="wq")
                wk = xw_pool.tile([P, cfg.kcd, cfg.qk_cols], BF16, tag="wk")
                wv = xw_pool.tile([P, cfg.kcd, cfg.qk_cols], BF16, tag="wv")
                for (w_sb, w_ext) in ((wq, wq_e), (wk, wk_e), (wv, wv_e)):
                    for kc in range(cfg.kcd):
                        nc.sync.dma_start(out=w_sb[:, kc, :],
                                          in_=w_ext[kc * P:(kc + 1) * P, :])

                # Q_T, K_T: [2 heads x 64, n] tiles per head pair, bf16
                for hp in range(cfg.hp):
                    qt_t = qk_pool.tile([P, cfg.n], BF16, tag=f"qT{hp}")
                    kt_t = qk_pool.tile([P, cfg.n], BF16, tag=f"kT{hp}")
                    qT.append(qt_t)
                    kT.append(kt_t)
                    for (dst, w_sb) in ((qt_t, wq), (kt_t, wk)):
                        for j in range(cfg.nstrip):
                            ps = psM.tile([P, STRIP], F32, tag="m")
                            for kc in range(cfg.kcd):
                                nc.tensor.matmul(
                                    ps[:],
                                    w_sb[:, kc, hp * P:(hp + 1) * P],
                                    xt[:, kc, j * STRIP:(j + 1) * STRIP],
                                    start=(kc == 0), stop=(kc == cfg.kcd - 1),
                                )
                            nc.vector.tensor_copy(
                                dst[:, j * STRIP:(j + 1) * STRIP], ps[:])

                # V: [k-chunk 128, hl*(64+1)] tiles (ones col for denominator)
                for kt_i in range(cfg.nkc):
                    vt = v_pool.tile([P, cfg.hl, 65], BF16, tag=f"v{kt_i}")
                    vsb.append(vt)
                    ps = psM.tile([P, cfg.qk_cols], F32, tag="m")
                    for kc in range(cfg.kcd):
                        nc.tensor.matmul(
                            ps[:],
                            xt[:, kc, kt_i * P:(kt_i + 1) * P],
                            wv[:, kc, :],
                            start=(kc == 0), stop=(kc == cfg.kcd - 1),
                        )
                    nc.vector.tensor_copy(
                        vt[:, :, 0:64],
                        ps[:].rearrange("p (h d) -> p h d", h=cfg.hl))
                    nc.vector.memset(vt[:, :, 64:65], 1.0)

            # ---------------- phase 2: attention + proj ----------------
            wp_sb = w2_pool.tile([P, cfg.hp, cfg.d_out], F32R, tag="wp")
            for hp in range(cfg.hp):
                nc.sync.dma_start(out=wp_sb[:, hp, :],
                                  in_=wp_e[hp * P:(hp + 1) * P, :])
            bp_sb = misc_pool.tile([P, cfg.oc], F32, tag="bp")
            nc.sync.dma_start(out=bp_sb[:], in_=bp_e[:])

            for j in range(cfg.nstrip):
                nkt = (j + 1) * cfg.kps
                o_tiles = []
                o_raw = []
                rec8 = rp_pool.tile([cfg.hl, STRIP], F32, tag="rec8")
                for h in range(cfg.hl):
                    hp, hh = h // 2, h % 2
                    if hh == 0:
                        ot = o_pool.tile([P, STRIP], F32R, tag=f"o{hp}")
                        o_tiles.append(ot)
                    po = psO.tile([65, STRIP], F32, tag="o")
                    eb4 = None
                    for pr in range(nkt // 2):
                        c0 = 2 * pr
                        qoff = P * (c0 - 4 * j) if c0 > 4 * j else 0
                        pss = psS.tile([P, 2, STRIP], F32, tag="s")
                        for sub in range(2):
                            kt_i = c0 + sub
                            nc.tensor.matmul(
                                pss[:, sub, qoff:],
                                kT[hp][hh * 64:(hh + 1) * 64,
                                       kt_i * P:(kt_i + 1) * P],
                                qT[hp][hh * 64:(hh + 1) * 64,
                                       j * STRIP + qoff:(j + 1) * STRIP],
                                start=True, stop=True,
                            )
                        if c0 % 4 == 0:
                            r0 = cfg.bias_off[j] + c0 * P
                            eb4 = eb_pool.tile([P, 4, STRIP], BF16, tag="eb")
                            nc.sync.dma_start(
                                out=eb4[:],
                                in_=bias_e[h, r0:r0 + 4 * P, :].rearrange(
                                    "(p c) q -> p c q", p=P))
                        p2 = p_pool.tile([P, 2, STRIP], BF16, tag="p")
                        nc.scalar.activation(p2[:, :, qoff:],
                                             pss[:, :, qoff:], Exp)
                        nc.vector.tensor_mul(
                            p2[:, :, qoff:], p2[:, :, qoff:],
                            eb4[:, (c0 % 4):(c0 % 4) + 2, qoff:])
                        for sub in range(2):
                            kt_i = c0 + sub
                            nc.tensor.matmul(
                                po[:, qoff:], vsb[kt_i][:, h, :],
                                p2[:, sub, qoff:],
                                start=(kt_i == 0), stop=(kt_i == nkt - 1),
                            )
                    orh = or_pool.tile([65, STRIP], F32, tag=f"or{h}")
                    o_raw.append(orh)
                    nc.vector.tensor_copy(orh[:], po[:])
                    nc.sync.dma_start(out=rec8[h:h + 1, :],
                                      in_=orh[64:65, :])
                # batched softmax division for all local heads of this strip
                with nc.allow_low_precision(reason="recip of f32 denom"):
                    nc.vector.reciprocal(rec8[:], rec8[:])
                nc.sync.dma_start(out=rec_dram[j][:], in_=rec8[:])
                for h in range(cfg.hl):
                    hp, hh = h // 2, h % 2
                    bch = bc_pool.tile([64, 1, STRIP], F32, tag="bc")
                    nc.sync.dma_start(
                        out=bch[:],
                        in_=rec_dram[j][h:h + 1, :].partition_broadcast(64))
                    nc.vector.tensor_mul(o_tiles[hp][hh * 64:(hh + 1) * 64, :],
                                         o_raw[h][0:64, :], bch[:, 0, :])

                # output projection for this strip (partial over local heads),
                # with b_proj/2 folded in (pair RS sums it back to b_proj)
                for oc in range(cfg.oc):
                    pp = psM.tile([P, STRIP], F32, tag="m")
                    for hp2 in range(cfg.hp):
                        nc.tensor.matmul(
                            pp[:],
                            wp_sb[:, hp2, oc * P:(oc + 1) * P],
                            o_tiles[hp2][:],
                            start=(hp2 == 0), stop=(hp2 == cfg.hp - 1),
                        )
                    pj_sb = fin_pool.tile([P, STRIP], F32, tag="pjsb")
                    nc.vector.tensor_scalar_add(pj_sb[:], pp[:],
                                                bp_sb[:, oc:oc + 1])
                    nc.sync.dma_start(out=partialT[j][oc * P:(oc + 1) * P, :],
                                      in_=pj_sb[:])

                nc.gpsimd.collective_compute(
                    "ReduceScatter",
                    mybir.AluOpType.add,
                    replica_groups=PAIRS,
                    ins=[partialT[j][:].opt()],
                    outs=[rs_out[j][:].opt()],
                )
                nc.sync.dma_start(
                    out=out_e[:, j * STRIP:(j + 1) * STRIP],
                    in_=rs_out[j][:])

    nc.compile()
    return nc


# ---------------- host-side shard / unshard ----------------

HEADS = 16
HEAD_DIM = 64
SCALE = HEAD_DIM ** -0.5


def make_in_maps(cfg: Cfg, x, pos_bias, w_qkv, w_proj, b_proj, n_cores=8):
    """Build per-core input dicts from full inputs."""
    x = np.asarray(x, dtype=np.float32)
    pos_bias = np.asarray(pos_bias, dtype=np.float32)
    w_qkv = np.asarray(w_qkv, dtype=np.float32)
    w_proj = np.asarray(w_proj, dtype=np.float32)
    b_proj = np.asarray(b_proj, dtype=np.float32)

    d = cfg.d_in
    hl = cfg.hl
    S = cfg.STRIP
    # causal mask pattern for a diagonal [S, S] block in [k, q] layout:
    # valid iff k_local <= q_local
    kl = np.arange(S)[:, None]
    ql = np.arange(S)[None, :]
    diag_valid = kl <= ql

    in_maps = []
    for c in range(n_cores):
        b = c // 2
        h0 = (c % 2) * hl
        cols = slice(h0 * cfg.dh, h0 * cfg.dh + cfg.qk_cols)
        xT = np.ascontiguousarray(x[b].T).astype(ml_dtypes.bfloat16)
        wq = np.ascontiguousarray(
            w_qkv[:, 0:d][:, cols] * SCALE).astype(ml_dtypes.bfloat16)
        wk = np.ascontiguousarray(
            w_qkv[:, d:2 * d][:, cols]).astype(ml_dtypes.bfloat16)
        wv = np.ascontiguousarray(
            w_qkv[:, 2 * d:3 * d][:, cols]).astype(ml_dtypes.bfloat16)
        wp = np.ascontiguousarray(w_proj[cols, :])
        # b_proj/2 per out-feature chunk (column oc <-> rows oc*128..+128)
        bp = np.ascontiguousarray((b_proj * 0.5).reshape(cfg.oc, cfg.P).T)

        # exp(bias) pack: the kernel computes P = exp(S) * exp(bias);
        # causally-masked entries get exp(bias) = 0.  Rows within each
        # quad of key-chunks are interleaved (p-major, chunk-minor) so a
        # whole [128, 4, 512] quad is one contiguous DMA.
        bias = np.empty((hl, cfg.bias_rows, S), dtype=ml_dtypes.bfloat16)
        for hh in range(hl):
            bT = pos_bias[h0 + hh].T  # [k, q]
            for j in range(cfg.nstrip):
                rows = S * (j + 1)
                blk = np.exp(bT[0:rows, j * S:(j + 1) * S])
                # mask the diagonal block (k > q invalid)
                dblk = blk[j * S:(j + 1) * S, :]
                dblk[~diag_valid] = 0.0
                nq = rows // 512
                blk = blk.reshape(nq, 4, cfg.P, S).transpose(0, 2, 1, 3)
                bias[hh, cfg.bias_off[j]:cfg.bias_off[j] + rows, :] = \
                    blk.reshape(rows, S)

        in_maps.append({
            "xT": xT, "wq": wq, "wk": wk, "wv": wv,
            "bias": bias, "wproj": wp, "bp": bp,
        })
    return in_maps


def assemble(cfg: Cfg, results, B=4):
    """Gather per-core [rs_rows, n] outputs into [B, n, d_out]."""
    out = np.empty((B, cfg.n, cfg.d_out), dtype=np.float32)
    for c in range(len(results)):
        b = c // 2
        rank = c % 2
        out[b, :, rank * cfg.rs_rows:(rank + 1) * cfg.rs_rows] = \
            results[c]["out"].T
    return out


_NC_CACHE = {}


def kernel(x, pos_bias, w_qkv, w_proj, b_proj):
    cfg = Cfg()
    if "nc" not in _NC_CACHE:
        _NC_CACHE["nc"] = build_nc(cfg)
    nc = _NC_CACHE["nc"]
    in_maps = make_in_maps(cfg, x, pos_bias, w_qkv, w_proj, b_proj)
    res = run_bass_kernel_spmd(nc, in_maps, core_ids=list(range(8)))
    return assemble(cfg, res.results)


# revision 15
# speedup vs baseline: 1.0481x; 1.0481x over previous
"""Trainium2 Bass kernel for causal multi-head attention with positional bias.

Reference computation (B=4, N=2048, D=1024, H=16, dh=64):
    qkv = x @ w_qkv; q,k,v = split(qkv); q *= dh**-0.5
    sim = q @ k.T + pos_bias; causal mask; attn = softmax(sim)
    out = (attn @ v) @ w_proj + b_proj

Sharding over 8 NeuronCores: core c handles batch c//2 and heads
8*(c%2) .. 8*(c%2)+8.  Each core computes its heads' full causal
attention in transposed layout (S_T[k,q] = K @ Q^T) so no on-chip
transposes are needed, then the partial output projection; a pair-wise
ReduceScatter (cores 2b, 2b+1) sums the head-sharded projection and
splits the output-feature dim.

Tricks:
- exp(bias) is precomputed on the host (masked entries = 0), so the bias
  add + causal mask become one bf16 multiply: P = exp(S) * EB.
- V carries an appended ones column, so the AV matmul also produces the
  softmax denominator for free.
- Softmax division: batched reciprocal of all 8 head denominators, then
  a partition-broadcast DMA (via DRAM) and one multiply per head.
- b_proj/2 is folded into the projection's PSUM->SBUF copy; after the
  pair ReduceScatter the halves sum to exactly b_proj.
- Causal width trimming: blocks above the diagonal are skipped entirely;
  the diagonal pair of key chunks only computes the valid query suffix.

Self-contained: hardcodes all shapes; no file reads.
"""

import numpy as np
import ml_dtypes

import concourse.bass as bass
import concourse.tile as tile
from concourse import bacc, mybir
from concourse.bass_utils import run_bass_kernel_spmd

F32 = mybir.dt.float32
F32R = mybir.dt.float32r
BF16 = mybir.dt.bfloat16

PAIRS = [[0, 1], [2, 3], [4, 5], [6, 7]]


class Cfg:
    """Geometry. Defaults = the real problem; small variants for sim tests."""

    def __init__(self, n=2048, d_in=1024, hl=8, d_out=1024):
        self.P = 128
        self.STRIP = 512          # query columns per strip (psum bank width)
        self.n = n                # sequence length
        self.d_in = d_in          # model dim
        self.hl = hl              # local heads per core
        self.dh = 64              # head dim
        self.d_out = d_out        # proj output dim
        self.nstrip = n // self.STRIP
        self.nkc = n // self.P            # key chunks of 128
        self.kcd = d_in // self.P         # contraction chunks over d_in
        self.hp = hl // 2                 # head pairs
        self.qk_cols = hl * self.dh       # local q (or k, v) columns
        self.oc = d_out // self.P         # out-feature chunks
        self.rs_rows = d_out // 2         # rows each core owns after RS
        kps = self.STRIP // self.P        # 128-row chunks per strip (4)
        self.kps = kps
        self.bias_off = [0] * self.nstrip
        off = 0
        for j in range(self.nstrip):
            self.bias_off[j] = off
            off += (j + 1) * self.STRIP
        self.bias_rows = off              # 5120 for full size


def build_nc(cfg: Cfg, num_devices=8, debug=False):
    P, STRIP = cfg.P, cfg.STRIP
    nc = bacc.Bacc("TRN2", target_bir_lowering=False, debug=debug,
                   num_devices=num_devices)

    xT_e = nc.dram_tensor("xT", [cfg.d_in, cfg.n], BF16, kind="ExternalInput")
    wq_e = nc.dram_tensor("wq", [cfg.d_in, cfg.qk_cols], BF16, kind="ExternalInput")
    wk_e = nc.dram_tensor("wk", [cfg.d_in, cfg.qk_cols], BF16, kind="ExternalInput")
    wv_e = nc.dram_tensor("wv", [cfg.d_in, cfg.qk_cols], BF16, kind="ExternalInput")
    bias_e = nc.dram_tensor("bias", [cfg.hl, cfg.bias_rows, STRIP], BF16,
                            kind="ExternalInput")
    wp_e = nc.dram_tensor("wproj", [cfg.qk_cols, cfg.d_out], F32R,
                          kind="ExternalInput")
    bp_e = nc.dram_tensor("bp", [P, cfg.oc], F32, kind="ExternalInput")
    out_e = nc.dram_tensor("out", [cfg.rs_rows, cfg.n], F32, kind="ExternalOutput")

    partialT = [nc.dram_tensor(f"partialT{j}", [cfg.d_out, STRIP], F32)
                for j in range(cfg.nstrip)]
    rs_out = [nc.dram_tensor(f"rs_out{j}", [cfg.rs_rows, STRIP], F32)
              for j in range(cfg.nstrip)]
    rec_dram = [nc.dram_tensor(f"rec{j}", [cfg.hl, STRIP], F32)
                for j in range(cfg.nstrip)]

    Exp = mybir.ActivationFunctionType.Exp

    with tile.TileContext(nc) as tc:
        with (
            tc.tile_pool(name="qk", bufs=1) as qk_pool,
            tc.tile_pool(name="vp", bufs=1) as v_pool,
            tc.tile_pool(name="eb", bufs=3) as eb_pool,
            tc.tile_pool(name="pp", bufs=6) as p_pool,
            tc.tile_pool(name="oo", bufs=2) as o_pool,
            tc.tile_pool(name="orw", bufs=1) as or_pool,
            tc.tile_pool(name="w2", bufs=1) as w2_pool,
            tc.tile_pool(name="misc", bufs=1) as misc_pool,
            tc.tile_pool(name="rp", bufs=2) as rp_pool,
            tc.tile_pool(name="bcp", bufs=2) as bc_pool,
            tc.tile_pool(name="fin", bufs=3) as fin_pool,
            tc.tile_pool(name="psM", bufs=2, space="PSUM") as psM,
            tc.tile_pool(name="psS", bufs=2, space="PSUM") as psS,
            tc.tile_pool(name="psO", bufs=2, space="PSUM") as psO,
        ):
            # ---------------- phase 1: QKV ----------------
            qT = []
            kT = []
            vsb = []
            with tc.tile_pool(name="xw", bufs=1) as xw_pool:
                xt = xw_pool.tile([P, cfg.kcd, cfg.n], BF16, tag="xt")
                for kc in range(cfg.kcd):
                    nc.sync.dma_start(out=xt[:, kc, :],
                                      in_=xT_e[kc * P:(kc + 1) * P, :])
                wq = xw_pool.tile([P, cfg.kcd, cfg.qk_cols], BF16, tag="wq")
                wk = xw_pool.tile([P, cfg.kcd, cfg.qk_cols], BF16, tag="wk")
                wv = xw_pool.tile([P, cfg.kcd, cfg.qk_cols], BF16, tag="wv")
                for (w_sb, w_ext) in ((wq, wq_e), (wk, wk_e), (wv, wv_e)):
                    for kc in range(cfg.kcd):
                        nc.sync.dma_start(out=w_sb[:, kc, :],
                                          in_=w_ext[kc * P:(kc + 1) * P, :])

                # Q_T: [2 heads x 64, n] tiles per head pair, bf16.
                # K_T: one zero-padded [128, n] tile per head ([K_h; 0] for
                # even heads, [0; K_h] for odd) so S matmuls stream the full
                # 128-partition contraction (keeps the PE HAM clock warm);
                # the zero half multiplies the other head's Q and adds 0.
                for h in range(cfg.hl):
                    ktp = qk_pool.tile([P, cfg.n], BF16, tag=f"kTp{h}")
                    kT.append(ktp)
                    z0 = (1 - h % 2) * 64
                    nc.vector.memset(ktp[z0:z0 + 64, :], 0.0)
                for hp in range(cfg.hp):
                    qt_t = qk_pool.tile([P, cfg.n], BF16, tag=f"qT{hp}")
                    qT.append(qt_t)
                    for j in range(cfg.nstrip):
                        ps = psM.tile([P, STRIP], F32, tag="m")
                        for kc in range(cfg.kcd):
                            nc.tensor.matmul(
                                ps[:],
                                wq[:, kc, hp * P:(hp + 1) * P],
                                xt[:, kc, j * STRIP:(j + 1) * STRIP],
                                start=(kc == 0), stop=(kc == cfg.kcd - 1),
                            )
                        nc.vector.tensor_copy(
                            qt_t[:, j * STRIP:(j + 1) * STRIP], ps[:])
                    for j in range(cfg.nstrip):
                        ps = psM.tile([P, STRIP], F32, tag="m")
                        for kc in range(cfg.kcd):
                            nc.tensor.matmul(
                                ps[:],
                                wk[:, kc, hp * P:(hp + 1) * P],
                                xt[:, kc, j * STRIP:(j + 1) * STRIP],
                                start=(kc == 0), stop=(kc == cfg.kcd - 1),
                            )
                        for hh in range(2):
                            h = 2 * hp + hh
                            nc.vector.tensor_copy(
                                kT[h][hh * 64:(hh + 1) * 64,
                                      j * STRIP:(j + 1) * STRIP],
                                ps[hh * 64:(hh + 1) * 64, :])

                # V: [k-chunk 128, hl*(64+1)] tiles (ones col for denominator)
                for kt_i in range(cfg.nkc):
                    vt = v_pool.tile([P, cfg.hl, 65], BF16, tag=f"v{kt_i}")
                    vsb.append(vt)
                    ps = psM.tile([P, cfg.qk_cols], F32, tag="m")
                    for kc in range(cfg.kcd):
                        nc.tensor.matmul(
                            ps[:],
                            xt[:, kc, kt_i * P:(kt_i + 1) * P],
                            wv[:, kc, :],
                            start=(kc == 0), stop=(kc == cfg.kcd - 1),
                        )
                    nc.vector.tensor_copy(
                        vt[:, :, 0:64],
                        ps[:].rearrange("p (h d) -> p h d", h=cfg.hl))
                    nc.vector.memset(vt[:, :, 64:65], 1.0)

            # ---------------- phase 2: attention + proj ----------------
            wp_sb = w2_pool.tile([P, cfg.hp, cfg.d_out], F32R, tag="wp")
            for hp in range(cfg.hp):
                nc.sync.dma_start(out=wp_sb[:, hp, :],
                                  in_=wp_e[hp * P:(hp + 1) * P, :])
            bp_sb = misc_pool.tile([P, cfg.oc], F32, tag="bp")
            nc.sync.dma_start(out=bp_sb[:], in_=bp_e[:])

            for j in range(cfg.nstrip):
                nkt = (j + 1) * cfg.kps
                o_tiles = []
                o_raw = []
                rec8 = rp_pool.tile([cfg.hl, STRIP], F32, tag="rec8")
                for h in range(cfg.hl):
                    hp, hh = h // 2, h % 2
                    if hh == 0:
                        ot = o_pool.tile([P, STRIP], F32R, tag=f"o{hp}")
                        o_tiles.append(ot)
                    po = psO.tile([65, STRIP], F32, tag="o")
                    eb4 = None
                    for pr in range(nkt // 2):
                        c0 = 2 * pr
                        qoff = P * (c0 - 4 * j) if c0 > 4 * j else 0
                        pss = psS.tile([P, 2, STRIP], F32, tag="s")
                        for sub in range(2):
                            kt_i = c0 + sub
                            nc.tensor.matmul(
                                pss[:, sub, qoff:],
                                kT[h][:, kt_i * P:(kt_i + 1) * P],
                                qT[hp][:, j * STRIP + qoff:(j + 1) * STRIP],
                                start=True, stop=True,
                            )
                        if c0 % 4 == 0:
                            r0 = cfg.bias_off[j] + c0 * P
                            eb4 = eb_pool.tile([P, 4, STRIP], BF16, tag="eb")
                            nc.sync.dma_start(
                                out=eb4[:],
                                in_=bias_e[h, r0:r0 + 4 * P, :].rearrange(
                                    "(p c) q -> p c q", p=P))
                        p2 = p_pool.tile([P, 2, STRIP], BF16, tag="p")
                        nc.scalar.activation(p2[:, :, qoff:],
                                             pss[:, :, qoff:], Exp)
                        nc.vector.tensor_mul(
                            p2[:, :, qoff:], p2[:, :, qoff:],
                            eb4[:, (c0 % 4):(c0 % 4) + 2, qoff:])
                        for sub in range(2):
                            kt_i = c0 + sub
                            nc.tensor.matmul(
                                po[:, qoff:], vsb[kt_i][:, h, :],
                                p2[:, sub, qoff:],
                                start=(kt_i == 0), stop=(kt_i == nkt - 1),
                            )
                    orh = or_pool.tile([65, STRIP], F32, tag=f"or{h}")
                    o_raw.append(orh)
                    nc.vector.tensor_copy(orh[:], po[:])
                    nc.sync.dma_start(out=rec8[h:h + 1, :],
                                      in_=orh[64:65, :])
                # batched softmax division for all local heads of this strip
                with nc.allow_low_precision(reason="recip of f32 denom"):
                    nc.vector.reciprocal(rec8[:], rec8[:])
                nc.sync.dma_start(out=rec_dram[j][:], in_=rec8[:])
                for h in range(cfg.hl):
                    hp, hh = h // 2, h % 2
                    bch = bc_pool.tile([64, 1, STRIP], F32, tag="bc")
                    nc.sync.dma_start(
                        out=bch[:],
                        in_=rec_dram[j][h:h + 1, :].partition_broadcast(64))
                    nc.vector.tensor_mul(o_tiles[hp][hh * 64:(hh + 1) * 64, :],
                                         o_raw[h][0:64, :], bch[:, 0, :])

                # output projection for this strip (partial over local heads),
                # with b_proj/2 folded in (pair RS sums it back to b_proj)
                for oc in range(cfg.oc):
                    pp = psM.tile([P, STRIP], F32, tag="m")
                    for hp2 in range(cfg.hp):
                        nc.tensor.matmul(
                            pp[:],
                            wp_sb[:, hp2, oc * P:(oc + 1) * P],
                            o_tiles[hp2][:],
                            start=(hp2 == 0), stop=(hp2 == cfg.hp - 1),
                        )
                    pj_sb = fin_pool.tile([P, STRIP], F32, tag="pjsb")
                    nc.vector.tensor_scalar_add(pj_sb[:], pp[:],
                                                bp_sb[:, oc:oc + 1])
                    nc.sync.dma_start(out=partialT[j][oc * P:(oc + 1) * P, :],
                                      in_=pj_sb[:])

                nc.gpsimd.collective_compute(
                    "ReduceScatter",
                    mybir.AluOpType.add,
                    replica_groups=PAIRS,
                    ins=[partialT[j][:].opt()],
                    outs=[rs_out[j][:].opt()],
                )
                nc.sync.dma_start(
                    out=out_e[:, j * STRIP:(j + 1) * STRIP],
                    in_=rs_out[j][:])

    nc.compile()
    return nc


# ---------------- host-side shard / unshard ----------------

HEADS = 16
HEAD_DIM = 64
SCALE = HEAD_DIM ** -0.5


def make_in_maps(cfg: Cfg, x, pos_bias, w_qkv, w_proj, b_proj, n_cores=8):
    """Build per-core input dicts from full inputs."""
    x = np.asarray(x, dtype=np.float32)
    pos_bias = np.asarray(pos_bias, dtype=np.float32)
    w_qkv = np.asarray(w_qkv, dtype=np.float32)
    w_proj = np.asarray(w_proj, dtype=np.float32)
    b_proj = np.asarray(b_proj, dtype=np.float32)

    d = cfg.d_in
    hl = cfg.hl
    S = cfg.STRIP
    # causal mask pattern for a diagonal [S, S] block in [k, q] layout:
    # valid iff k_local <= q_local
    kl = np.arange(S)[:, None]
    ql = np.arange(S)[None, :]
    diag_valid = kl <= ql

    in_maps = []
    for c in range(n_cores):
        b = c // 2
        h0 = (c % 2) * hl
        cols = slice(h0 * cfg.dh, h0 * cfg.dh + cfg.qk_cols)
        xT = np.ascontiguousarray(x[b].T).astype(ml_dtypes.bfloat16)
        wq = np.ascontiguousarray(
            w_qkv[:, 0:d][:, cols] * SCALE).astype(ml_dtypes.bfloat16)
        wk = np.ascontiguousarray(
            w_qkv[:, d:2 * d][:, cols]).astype(ml_dtypes.bfloat16)
        wv = np.ascontiguousarray(
            w_qkv[:, 2 * d:3 * d][:, cols]).astype(ml_dtypes.bfloat16)
        wp = np.ascontiguousarray(w_proj[cols, :])
        # b_proj/2 per out-feature chunk (column oc <-> rows oc*128..+128)
        bp = np.ascontiguousarray((b_proj * 0.5).reshape(cfg.oc, cfg.P).T)

        # exp(bias) pack: the kernel computes P = exp(S) * exp(bias);
        # causally-masked entries get exp(bias) = 0.  Rows within each
        # quad of key-chunks are interleaved (p-major, chunk-minor) so a
        # whole [128, 4, 512] quad is one contiguous DMA.
        bias = np.empty((hl, cfg.bias_rows, S), dtype=ml_dtypes.bfloat16)
        for hh in range(hl):
            bT = pos_bias[h0 + hh].T  # [k, q]
            for j in range(cfg.nstrip):
                rows = S * (j + 1)
                blk = np.exp(bT[0:rows, j * S:(j + 1) * S])
                # mask the diagonal block (k > q invalid)
                dblk = blk[j * S:(j + 1) * S, :]
                dblk[~diag_valid] = 0.0
                nq = rows // 512
                blk = blk.reshape(nq, 4, cfg.P, S).transpose(0, 2, 1, 3)
                bias[hh, cfg.bias_off[j]:cfg.bias_off[j] + rows, :] = \
                    blk.reshape(rows, S)

        in_maps.append({
            "xT": xT, "wq": wq, "wk": wk, "wv": wv,
            "bias": bias, "wproj": wp, "bp": bp,
        })
    return in_maps


def assemble(cfg: Cfg, results, B=4):
    """Gather per-core [rs_rows, n] outputs into [B, n, d_out]."""
    out = np.empty((B, cfg.n, cfg.d_out), dtype=np.float32)
    for c in range(len(results)):
        b = c // 2
        rank = c % 2
        out[b, :, rank * cfg.rs_rows:(rank + 1) * cfg.rs_rows] = \
            results[c]["out"].T
    return out


_NC_CACHE = {}


def kernel(x, pos_bias, w_qkv, w_proj, b_proj):
    cfg = Cfg()
    if "nc" not in _NC_CACHE:
        _NC_CACHE["nc"] = build_nc(cfg)
    nc = _NC_CACHE["nc"]
    in_maps = make_in_maps(cfg, x, pos_bias, w_qkv, w_proj, b_proj)
    res = run_bass_kernel_spmd(nc, in_maps, core_ids=list(range(8)))
    return assemble(cfg, res.results)


# revision 16
# speedup vs baseline: 1.0543x; 1.0059x over previous
"""Trainium2 Bass kernel for causal multi-head attention with positional bias.

Reference computation (B=4, N=2048, D=1024, H=16, dh=64):
    qkv = x @ w_qkv; q,k,v = split(qkv); q *= dh**-0.5
    sim = q @ k.T + pos_bias; causal mask; attn = softmax(sim)
    out = (attn @ v) @ w_proj + b_proj

Sharding over 8 NeuronCores: core c handles batch c//2 and heads
8*(c%2) .. 8*(c%2)+8.  Each core computes its heads' full causal
attention in transposed layout (S_T[k,q] = K @ Q^T) so no on-chip
transposes are needed, then the partial output projection; a pair-wise
ReduceScatter (cores 2b, 2b+1) sums the head-sharded projection and
splits the output-feature dim.

Tricks:
- exp(bias) is precomputed on the host (masked entries = 0), so the bias
  add + causal mask become one bf16 multiply: P = exp(S) * EB.
- V carries an appended ones column, so the AV matmul also produces the
  softmax denominator for free.
- Softmax division: batched reciprocal of all 8 head denominators, then
  a partition-broadcast DMA (via DRAM) and one multiply per head.
- b_proj/2 is folded into the projection's PSUM->SBUF copy; after the
  pair ReduceScatter the halves sum to exactly b_proj.
- Causal width trimming: blocks above the diagonal are skipped entirely;
  the diagonal pair of key chunks only computes the valid query suffix.

Self-contained: hardcodes all shapes; no file reads.
"""

import numpy as np
import ml_dtypes

import concourse.bass as bass
import concourse.tile as tile
from concourse import bacc, mybir
from concourse.bass_utils import run_bass_kernel_spmd

F32 = mybir.dt.float32
F32R = mybir.dt.float32r
BF16 = mybir.dt.bfloat16

PAIRS = [[0, 1], [2, 3], [4, 5], [6, 7]]


class Cfg:
    """Geometry. Defaults = the real problem; small variants for sim tests."""

    def __init__(self, n=2048, d_in=1024, hl=8, d_out=1024):
        self.P = 128
        self.STRIP = 512          # query columns per strip (psum bank width)
        self.n = n                # sequence length
        self.d_in = d_in          # model dim
        self.hl = hl              # local heads per core
        self.dh = 64              # head dim
        self.d_out = d_out        # proj output dim
        self.nstrip = n // self.STRIP
        self.nkc = n // self.P            # key chunks of 128
        self.kcd = d_in // self.P         # contraction chunks over d_in
        self.hp = hl // 2                 # head pairs
        self.qk_cols = hl * self.dh       # local q (or k, v) columns
        self.oc = d_out // self.P         # out-feature chunks
        self.rs_rows = d_out // 2         # rows each core owns after RS
        kps = self.STRIP // self.P        # 128-row chunks per strip (4)
        self.kps = kps
        self.bias_off = [0] * self.nstrip
        off = 0
        for j in range(self.nstrip):
            self.bias_off[j] = off
            off += (j + 1) * self.STRIP
        self.bias_rows = off              # 5120 for full size


def build_nc(cfg: Cfg, num_devices=8, debug=False):
    P, STRIP = cfg.P, cfg.STRIP
    nc = bacc.Bacc("TRN2", target_bir_lowering=False, debug=debug,
                   num_devices=num_devices)

    xT_e = nc.dram_tensor("xT", [cfg.d_in, cfg.n], BF16, kind="ExternalInput")
    wq_e = nc.dram_tensor("wq", [cfg.d_in, cfg.qk_cols], BF16, kind="ExternalInput")
    wk_e = nc.dram_tensor("wk", [cfg.d_in, cfg.qk_cols], BF16, kind="ExternalInput")
    wv_e = nc.dram_tensor("wv", [cfg.d_in, cfg.qk_cols], BF16, kind="ExternalInput")
    bias_e = nc.dram_tensor("bias", [cfg.hl, cfg.bias_rows, STRIP], BF16,
                            kind="ExternalInput")
    wp_e = nc.dram_tensor("wproj", [cfg.qk_cols, cfg.d_out], F32R,
                          kind="ExternalInput")
    bp_e = nc.dram_tensor("bp", [P, cfg.oc], F32, kind="ExternalInput")
    out_e = nc.dram_tensor("out", [cfg.rs_rows, cfg.n], F32, kind="ExternalOutput")

    partialT = [nc.dram_tensor(f"partialT{j}", [cfg.d_out, STRIP], F32)
                for j in range(cfg.nstrip)]
    rs_out = [nc.dram_tensor(f"rs_out{j}", [cfg.rs_rows, STRIP], F32)
              for j in range(cfg.nstrip)]
    rec_dram = [nc.dram_tensor(f"rec{j}", [cfg.hl, STRIP], F32)
                for j in range(cfg.nstrip)]

    Exp = mybir.ActivationFunctionType.Exp

    with tile.TileContext(nc) as tc:
        with (
            tc.tile_pool(name="qk", bufs=1) as qk_pool,
            tc.tile_pool(name="vp", bufs=1) as v_pool,
            tc.tile_pool(name="eb", bufs=3) as eb_pool,
            tc.tile_pool(name="pp", bufs=6) as p_pool,
            tc.tile_pool(name="oo", bufs=2) as o_pool,
            tc.tile_pool(name="orw", bufs=1) as or_pool,
            tc.tile_pool(name="w2", bufs=1) as w2_pool,
            tc.tile_pool(name="misc", bufs=1) as misc_pool,
            tc.tile_pool(name="rp", bufs=2) as rp_pool,
            tc.tile_pool(name="bcp", bufs=2) as bc_pool,
            tc.tile_pool(name="fin", bufs=3) as fin_pool,
            tc.tile_pool(name="psM", bufs=2, space="PSUM") as psM,
            tc.tile_pool(name="psS", bufs=2, space="PSUM") as psS,
            tc.tile_pool(name="psO", bufs=2, space="PSUM") as psO,
        ):
            # ---------------- phase 1: QKV ----------------
            qT = []
            kT = []
            vsb = []
            with tc.tile_pool(name="xw", bufs=1) as xw_pool:
                xt = xw_pool.tile([P, cfg.kcd, cfg.n], BF16, tag="xt")
                for kc in range(cfg.kcd):
                    nc.sync.dma_start(out=xt[:, kc, :],
                                      in_=xT_e[kc * P:(kc + 1) * P, :])
                wq = xw_pool.tile([P, cfg.kcd, cfg.qk_cols], BF16, tag="wq")
                wk = xw_pool.tile([P, cfg.kcd, cfg.qk_cols], BF16, tag="wk")
                wv = xw_pool.tile([P, cfg.kcd, cfg.qk_cols], BF16, tag="wv")
                for (w_sb, w_ext) in ((wq, wq_e), (wk, wk_e), (wv, wv_e)):
                    for kc in range(cfg.kcd):
                        nc.sync.dma_start(out=w_sb[:, kc, :],
                                          in_=w_ext[kc * P:(kc + 1) * P, :])

                # Q_T: [2 heads x 64, n] tiles per head pair, bf16.
                # K_T: one zero-padded [128, n] tile per head ([K_h; 0] for
                # even heads, [0; K_h] for odd) so S matmuls stream the full
                # 128-partition contraction (keeps the PE HAM clock warm);
                # the zero half multiplies the other head's Q and adds 0.
                for h in range(cfg.hl):
                    ktp = qk_pool.tile([P, cfg.n], BF16, tag=f"kTp{h}")
                    kT.append(ktp)
                    z0 = (1 - h % 2) * 64
                    nc.vector.memset(ktp[z0:z0 + 64, :], 0.0)
                vsb.extend([None] * cfg.nkc)

                def emit_v(kt_i):
                    vt = v_pool.tile([P, cfg.hl, 65], BF16, tag=f"v{kt_i}")
                    vsb[kt_i] = vt
                    ps = psM.tile([P, cfg.qk_cols], F32, tag="m")
                    for kc in range(cfg.kcd):
                        nc.tensor.matmul(
                            ps[:],
                            xt[:, kc, kt_i * P:(kt_i + 1) * P],
                            wv[:, kc, :],
                            start=(kc == 0), stop=(kc == cfg.kcd - 1),
                        )
                    nc.vector.tensor_copy(
                        vt[:, :, 0:64],
                        ps[:].rearrange("p (h d) -> p h d", h=cfg.hl))
                    nc.vector.memset(vt[:, :, 64:65], 1.0)

                vg = cfg.nkc // cfg.hp
                for hp in range(cfg.hp):
                    qt_t = qk_pool.tile([P, cfg.n], BF16, tag=f"qT{hp}")
                    qT.append(qt_t)
                    for j in range(cfg.nstrip):
                        ps = psM.tile([P, STRIP], F32, tag="m")
                        for kc in range(cfg.kcd):
                            nc.tensor.matmul(
                                ps[:],
                                wq[:, kc, hp * P:(hp + 1) * P],
                                xt[:, kc, j * STRIP:(j + 1) * STRIP],
                                start=(kc == 0), stop=(kc == cfg.kcd - 1),
                            )
                        nc.vector.tensor_copy(
                            qt_t[:, j * STRIP:(j + 1) * STRIP], ps[:])
                    for j in range(cfg.nstrip):
                        ps = psM.tile([P, STRIP], F32, tag="m")
                        for kc in range(cfg.kcd):
                            nc.tensor.matmul(
                                ps[:],
                                wk[:, kc, hp * P:(hp + 1) * P],
                                xt[:, kc, j * STRIP:(j + 1) * STRIP],
                                start=(kc == 0), stop=(kc == cfg.kcd - 1),
                            )
                        for hh in range(2):
                            h = 2 * hp + hh
                            nc.vector.tensor_copy(
                                kT[h][hh * 64:(hh + 1) * 64,
                                      j * STRIP:(j + 1) * STRIP],
                                ps[hh * 64:(hh + 1) * 64, :])
                    for kt_i in range(hp * vg, (hp + 1) * vg):
                        emit_v(kt_i)


            # ---------------- phase 2: attention + proj ----------------
            wp_sb = w2_pool.tile([P, cfg.hp, cfg.d_out], F32R, tag="wp")
            for hp in range(cfg.hp):
                nc.sync.dma_start(out=wp_sb[:, hp, :],
                                  in_=wp_e[hp * P:(hp + 1) * P, :])
            bp_sb = misc_pool.tile([P, cfg.oc], F32, tag="bp")
            nc.sync.dma_start(out=bp_sb[:], in_=bp_e[:])

            strip_state = {}

            def emit_epilogue(j):
                o_tiles, o_raw, rec8 = strip_state.pop(j)
                # batched softmax division for all local heads of this strip
                with nc.allow_low_precision(reason="recip of f32 denom"):
                    nc.vector.reciprocal(rec8[:], rec8[:])
                nc.sync.dma_start(out=rec_dram[j][:], in_=rec8[:])
                for h in range(cfg.hl):
                    hp, hh = h // 2, h % 2
                    bch = bc_pool.tile([64, 1, STRIP], F32, tag="bc")
                    nc.sync.dma_start(
                        out=bch[:],
                        in_=rec_dram[j][h:h + 1, :].partition_broadcast(64))
                    nc.vector.tensor_mul(o_tiles[hp][hh * 64:(hh + 1) * 64, :],
                                         o_raw[h][0:64, :], bch[:, 0, :])

                # output projection for this strip (partial over local heads),
                # with b_proj/2 folded in (pair RS sums it back to b_proj)
                for oc in range(cfg.oc):
                    pp = psM.tile([P, STRIP], F32, tag="m")
                    for hp2 in range(cfg.hp):
                        nc.tensor.matmul(
                            pp[:],
                            wp_sb[:, hp2, oc * P:(oc + 1) * P],
                            o_tiles[hp2][:],
                            start=(hp2 == 0), stop=(hp2 == cfg.hp - 1),
                        )
                    pj_sb = fin_pool.tile([P, STRIP], F32, tag="pjsb")
                    nc.vector.tensor_scalar_add(pj_sb[:], pp[:],
                                                bp_sb[:, oc:oc + 1])
                    nc.sync.dma_start(out=partialT[j][oc * P:(oc + 1) * P, :],
                                      in_=pj_sb[:])

                nc.gpsimd.collective_compute(
                    "ReduceScatter",
                    mybir.AluOpType.add,
                    replica_groups=PAIRS,
                    ins=[partialT[j][:].opt()],
                    outs=[rs_out[j][:].opt()],
                )
                nc.sync.dma_start(
                    out=out_e[:, j * STRIP:(j + 1) * STRIP],
                    in_=rs_out[j][:])

            for j in range(cfg.nstrip):
                nkt = (j + 1) * cfg.kps
                o_tiles = []
                o_raw = []
                rec8 = rp_pool.tile([cfg.hl, STRIP], F32, tag="rec8")
                for h in range(cfg.hl):
                    hp, hh = h // 2, h % 2
                    if hh == 0:
                        ot = o_pool.tile([P, STRIP], F32R, tag=f"o{hp}")
                        o_tiles.append(ot)
                    po = psO.tile([65, STRIP], F32, tag="o")
                    eb4 = None
                    for pr in range(nkt // 2):
                        c0 = 2 * pr
                        qoff = P * (c0 - 4 * j) if c0 > 4 * j else 0
                        pss = psS.tile([P, 2, STRIP], F32, tag="s")
                        for sub in range(2):
                            kt_i = c0 + sub
                            nc.tensor.matmul(
                                pss[:, sub, qoff:],
                                kT[h][:, kt_i * P:(kt_i + 1) * P],
                                qT[hp][:, j * STRIP + qoff:(j + 1) * STRIP],
                                start=True, stop=True,
                            )
                        if c0 % 4 == 0:
                            r0 = cfg.bias_off[j] + c0 * P
                            eb4 = eb_pool.tile([P, 4, STRIP], BF16, tag="eb")
                            nc.sync.dma_start(
                                out=eb4[:],
                                in_=bias_e[h, r0:r0 + 4 * P, :].rearrange(
                                    "(p c) q -> p c q", p=P))
                        p2 = p_pool.tile([P, 2, STRIP], BF16, tag="p")
                        nc.scalar.activation(p2[:, :, qoff:],
                                             pss[:, :, qoff:], Exp)
                        nc.vector.tensor_mul(
                            p2[:, :, qoff:], p2[:, :, qoff:],
                            eb4[:, (c0 % 4):(c0 % 4) + 2, qoff:])
                        for sub in range(2):
                            kt_i = c0 + sub
                            nc.tensor.matmul(
                                po[:, qoff:], vsb[kt_i][:, h, :],
                                p2[:, sub, qoff:],
                                start=(kt_i == 0), stop=(kt_i == nkt - 1),
                            )
                    orh = or_pool.tile([65, STRIP], F32, tag=f"or{h}")
                    o_raw.append(orh)
                    nc.vector.tensor_copy(orh[:], po[:])
                    nc.sync.dma_start(out=rec8[h:h + 1, :],
                                      in_=orh[64:65, :])
                    if h == 0 and j > 0:
                        # overlap the previous strip's epilogue with this
                        # strip's attention (keeps the PE stream dense)
                        emit_epilogue(j - 1)
                strip_state[j] = (o_tiles, o_raw, rec8)
            emit_epilogue(cfg.nstrip - 1)

    nc.compile()
    return nc


# ---------------- host-side shard / unshard ----------------

HEADS = 16
HEAD_DIM = 64
SCALE = HEAD_DIM ** -0.5


def make_in_maps(cfg: Cfg, x, pos_bias, w_qkv, w_proj, b_proj, n_cores=8):
    """Build per-core input dicts from full inputs."""
    x = np.asarray(x, dtype=np.float32)
    pos_bias = np.asarray(pos_bias, dtype=np.float32)
    w_qkv = np.asarray(w_qkv, dtype=np.float32)
    w_proj = np.asarray(w_proj, dtype=np.float32)
    b_proj = np.asarray(b_proj, dtype=np.float32)

    d = cfg.d_in
    hl = cfg.hl
    S = cfg.STRIP
    # causal mask pattern for a diagonal [S, S] block in [k, q] layout:
    # valid iff k_local <= q_local
    kl = np.arange(S)[:, None]
    ql = np.arange(S)[None, :]
    diag_valid = kl <= ql

    in_maps = []
    for c in range(n_cores):
        b = c // 2
        h0 = (c % 2) * hl
        cols = slice(h0 * cfg.dh, h0 * cfg.dh + cfg.qk_cols)
        xT = np.ascontiguousarray(x[b].T).astype(ml_dtypes.bfloat16)
        wq = np.ascontiguousarray(
            w_qkv[:, 0:d][:, cols] * SCALE).astype(ml_dtypes.bfloat16)
        wk = np.ascontiguousarray(
            w_qkv[:, d:2 * d][:, cols]).astype(ml_dtypes.bfloat16)
        wv = np.ascontiguousarray(
            w_qkv[:, 2 * d:3 * d][:, cols]).astype(ml_dtypes.bfloat16)
        wp = np.ascontiguousarray(w_proj[cols, :])
        # b_proj/2 per out-feature chunk (column oc <-> rows oc*128..+128)
        bp = np.ascontiguousarray((b_proj * 0.5).reshape(cfg.oc, cfg.P).T)

        # exp(bias) pack: the kernel computes P = exp(S) * exp(bias);
        # causally-masked entries get exp(bias) = 0.  Rows within each
        # quad of key-chunks are interleaved (p-major, chunk-minor) so a
        # whole [128, 4, 512] quad is one contiguous DMA.
        bias = np.empty((hl, cfg.bias_rows, S), dtype=ml_dtypes.bfloat16)
        for hh in range(hl):
            bT = pos_bias[h0 + hh].T  # [k, q]
            for j in range(cfg.nstrip):
                rows = S * (j + 1)
                blk = np.exp(bT[0:rows, j * S:(j + 1) * S])
                # mask the diagonal block (k > q invalid)
                dblk = blk[j * S:(j + 1) * S, :]
                dblk[~diag_valid] = 0.0
                nq = rows // 512
                blk = blk.reshape(nq, 4, cfg.P, S).transpose(0, 2, 1, 3)
                bias[hh, cfg.bias_off[j]:cfg.bias_off[j] + rows, :] = \
                    blk.reshape(rows, S)

        in_maps.append({
            "xT": xT, "wq": wq, "wk": wk, "wv": wv,
            "bias": bias, "wproj": wp, "bp": bp,
        })
    return in_maps


def assemble(cfg: Cfg, results, B=4):
    """Gather per-core [rs_rows, n] outputs into [B, n, d_out]."""
    out = np.empty((B, cfg.n, cfg.d_out), dtype=np.float32)
    for c in range(len(results)):
        b = c // 2
        rank = c % 2
        out[b, :, rank * cfg.rs_rows:(rank + 1) * cfg.rs_rows] = \
            results[c]["out"].T
    return out


_NC_CACHE = {}


def kernel(x, pos_bias, w_qkv, w_proj, b_proj):
    cfg = Cfg()
    if "nc" not in _NC_CACHE:
        _NC_CACHE["nc"] = build_nc(cfg)
    nc = _NC_CACHE["nc"]
    in_maps = make_in_maps(cfg, x, pos_bias, w_qkv, w_proj, b_proj)
    res = run_bass_kernel_spmd(nc, in_maps, core_ids=list(range(8)))
    return assemble(cfg, res.results)
